# revision 3
# baseline (speedup 1.0000x reference)
"""CompressedLinear (quantized low-rank linear) on 8 trn2 NeuronCores, v2.

y = ((x @ dequant(Vh).T) * dequant(S)) @ dequant(U).T + bias

Data-parallel over tokens (1024 tokens/core), weights replicated. Two chained
bf16 matmuls with fp32 PSUM accumulation. All dequant folds happen on host:
vh/u are stored as (int - zp) in bf16 (exact for 8-bit ints), the combined
scale (s_v*s_s*s_u)*(S - zp_s) is folded into the mm1 eviction, bias is added
during the mm2 eviction. Output leaves the device as bf16 (half the write
traffic); host widens to f32.

The tensor-engine stream is organized to minimize LDWEIGHTS (re)loads, the
main source of PE bubbles in v1 (1 load per matmul = ~46ns each):
 - mm1: two passes over rank halves with 4 two-bank PSUM tiles; each vh
   128x128 chunk is loaded once and used by 2 adjacent N=512 matmuls
   (256 loads / 512 matmuls).
 - mm2: token-major; each hT 128x128 tile is loaded once and used by 8
   adjacent N=512 matmuls across 4 two-bank PSUM tiles (64 loads / 512
   matmuls).
 - a post-schedule BIR pass drops InstLdweights that are identical to the
   previously loaded weights (Bass emits one per matmul unconditionally),
   merging their semaphore syncs into the following matmul.
"""

import os

import numpy as np
import ml_dtypes

IN_F, OUT_F, RANK = 4096, 4096, 1024
B, S_LEN = 4, 2048
N_CORES = 8
P = 128
NTOK = B * S_LEN            # 8192 tokens total
TPC = NTOK // N_CORES       # 1024 tokens per core
TBS = 512                   # tokens per matmul (one PSUM bank of fp32)
TB = TPC // TBS             # 2 token blocks
KO = IN_F // P              # 32 k-tiles (mm1 contraction)
RO = RANK // P              # 8 rank tiles
NT = TPC // P               # 8 token tiles (mm2 stationary)
RHALF = RO // 2             # 4 rank tiles per mm1 pass
NJ = 4                      # ob-pair PSUM tiles in mm2 (4 x 1024 outs)

_BF16 = ml_dtypes.bfloat16

last_run = None
_nc_cache = {}

# ko chunk sizes for streaming DMAs (first chunks small for fast start)
_XCH = [1, 3, 4, 8, 8, 8]
_VCH0 = [1, 3, 4, 8, 8, 8]
_VCH1 = [8, 8, 8, 8]


def _dedupe_ldweights(nc, mybir):
    """Remove InstLdweights whose weights AP matches the previously loaded
    one (Bass emits one per matmul). Sync waits/updates of a removed load
    are merged into the next matmul so no dependency is lost."""
    removed = 0
    for blk in nc.main_func.blocks:
        cur_sig = None
        pend_w, pend_u = [], []
        to_remove = []
        for ins in list(blk.instructions):
            if isinstance(ins, mybir.InstLdweights):
                sig = (str(ins.ins[0]), str(ins.perf_mode),
                       str(ins.is_transpose), str(ins.tile_position),
                       str(ins.tile_size))
                if sig == cur_sig:
                    si = ins.sync_info
                    if si is not None:
                        pend_w.extend(list(si.on_wait or []))
                        pend_u.extend(list(si.on_update or []))
                    to_remove.append(ins)
                    removed += 1
                else:
                    cur_sig = sig
                continue
            if isinstance(ins, mybir.InstMatmult):
                if ins.ldweights or ins.is_transpose:
                    cur_sig = None
                if pend_w or pend_u:
                    si = ins.sync_info
                    ow = list(si.on_wait or []) if si else []
                    ou = list(si.on_update or []) if si else []
                    ins.sync_info = mybir.SyncInfo(on_wait=ow + pend_w,
                                                   on_update=ou + pend_u)
                    pend_w, pend_u = [], []
        assert not (pend_w or pend_u), "dangling sync from removed ldweights"
        for ins in to_remove:
            blk.instructions.remove(ins)
    return removed


def _build_nc():
    import concourse.mybir as mybir
    import concourse.tile as tile
    from concourse import bacc

    f32 = mybir.dt.float32
    bf16 = mybir.dt.bfloat16
    OP = mybir.AluOpType
    ACT = mybir.ActivationFunctionType

    nc = bacc.Bacc("TRN2", target_bir_lowering=False, debug=False,
                   num_devices=N_CORES)

    # Host-packed inputs (see kernel() for the exact packing):
    #   xr[p, ko, blk, t]  = x_bf16[blk*512 + t, ko*128 + p]
    #   vhr[p, h, ko, r]   = (Vh.T - zp_v)[ko*128 + p, h*512 + r]    bf16
    #   ur[p, rk, o]       = (U.T - zp_u)[rk*128 + p, o]             bf16
    #   sc[p, rt]          = combined scale for rank rt*128 + p      f32
    xr = nc.dram_tensor("xr", [P, KO, TB, TBS], bf16, kind="ExternalInput")
    vhr = nc.dram_tensor("vhr", [P, 2, KO, TBS], bf16, kind="ExternalInput")
    ur = nc.dram_tensor("ur", [P, RO, OUT_F], bf16, kind="ExternalInput")
    sc = nc.dram_tensor("sc", [P, RO], f32, kind="ExternalInput")
    y = nc.dram_tensor("y", [TPC, OUT_F], bf16, kind="ExternalOutput")

    with tile.TileContext(nc) as tc:
        with (
            tc.tile_pool(name="const", bufs=1) as const,
            tc.tile_pool(name="vhp", bufs=3) as vhp,
            tc.tile_pool(name="yp", bufs=6) as yp,
            tc.tile_pool(name="psp", bufs=4, space="PSUM") as psp,
        ):
            # All inputs go on the Sync queue in exact consumption order so
            # no stream can starve another of HBM bandwidth: pass-A needs
            # vh(h0, ko) and x(ko) together at ~200 GB/s; everything later
            # (vh h1, bias, u) follows behind.
            x_sb = const.tile([P, KO, TB, TBS], bf16, name="x_sb")
            vh_t = {}

            def vh_chunk(h, ko, ch):
                vt = vhp.tile([P, ch, TBS], bf16, tag="vh",
                              name=f"vh_{h}_{ko}")
                nc.sync.dma_start(vt[:], vhr.ap()[:, h, ko:ko + ch, :])
                for k in range(ch):
                    vh_t[(h, ko + k)] = (vt, k)

            ko = 0
            for ch in _VCH0:                     # pass A: vh(h0) + x paired
                # x first: it's the larger transfer, so starting it first
                # minimizes the time until BOTH tiles of a ko are resident
                nc.sync.dma_start(x_sb[:, ko:ko + ch, :, :],
                                  xr.ap()[:, ko:ko + ch, :, :])
                vh_chunk(0, ko, ch)
                ko += ch
            s_sb = const.tile([P, RO], f32, name="s_sb")
            nc.sync.dma_start(s_sb[:], sc.ap())
            ko = 0
            for ch in _VCH1:                     # pass B: vh(h1)
                vh_chunk(1, ko, ch)
                ko += ch
            u_sb = const.tile([P, RO, OUT_F], bf16, name="u_sb")
            for rk in range(RO):
                nc.sync.dma_start(u_sb[:, rk, :], ur.ap()[:, rk, :])

            # hT[p, rk, blk, t] (bf16): mm1 result, rank on partitions.
            hT = const.tile([P, RO, TB, TBS], bf16, name="hT")

            # ---- mm1: two passes over rank halves; each vh 128x128 chunk
            # loaded once, used by two adjacent N=512 matmuls ----
            for h in range(2):
                pst = [psp.tile([P, TB, TBS], f32, tag="ps",
                                name=f"ps1_{h}_{i}") for i in range(RHALF)]
                for ko in range(KO):
                    vt, k = vh_t[(h, ko)]
                    for i in range(RHALF):
                        for blk in range(TB):
                            nc.tensor.matmul(
                                pst[i][:, blk, :],
                                vt[:, k, i * P:(i + 1) * P],
                                x_sb[:, ko, blk, :],
                                start=(ko == 0), stop=(ko == KO - 1))
                # evict: hT[rt] = psum * s_comb[rt] (per-partition scalar),
                # alternating DVE / ScalarE
                for i in range(RHALF):
                    rt = h * RHALF + i
                    if i % 2 == 0:
                        nc.vector.tensor_scalar(
                            hT[:, rt, :, :], pst[i][:],
                            s_sb[:, rt:rt + 1], None, OP.mult)
                    else:
                        nc.scalar.activation(
                            hT[:, rt, :, :], pst[i][:], ACT.Copy,
                            scale=s_sb[:, rt:rt + 1])

            # ---- mm2: token-major; groups of (token tile, out-half) use 4
            # PSUM banks each, so the 8-bank pool is a 2-group-deep ring and
            # evictions have a full group (~7us) of slack. Each hT 128x128
            # tile is loaded once per group and used by 4 adjacent N=512
            # matmuls (LDWEIGHTS hides completely at that ratio). ----
            for t in range(NT):
                blk, toff = t // (NT // TB), (t % (NT // TB)) * P
                # last token tile: single-tile groups so the final
                # evict+DMA chain after the last matmul is half as long
                ngrp, npsy = (2, 2) if t < NT - 1 else (4, 1)
                for oh in range(ngrp):
                    psy = [psp.tile([P, TB, TBS], f32, tag="ps",
                                    name=f"ps2_{t}_{oh}_{j}")
                           for j in range(npsy)]
                    for rk in range(RO):
                        lhsT = hT[:, rk, blk, toff:toff + P]
                        for j in range(npsy):
                            for half in range(2):
                                ob = npsy * 2 * oh + 2 * j + half
                                nc.tensor.matmul(
                                    psy[j][:, half, :],
                                    lhsT,
                                    u_sb[:, rk, ob * TBS:(ob + 1) * TBS],
                                    start=(rk == 0), stop=(rk == RO - 1))
                    for j in range(npsy):
                        # pure psum->bf16 copy (bias is added on the host in
                        # f32), one on DVE and one on ScalarE; the y DMA
                        # dispatch goes on the idle GpSimd queue so it never
                        # delays the ScalarE evicts.
                        yt = yp.tile([P, TB, TBS], bf16, tag="yt", name="yt")
                        if (j + oh) % 2 == 0:
                            nc.vector.tensor_scalar(
                                yt[:], psy[j][:], 1.0, None, OP.mult)
                        else:
                            nc.scalar.activation(yt[:], psy[j][:], ACT.Copy)
                        co = (npsy * oh + j) * 2 * TBS
                        nc.gpsimd.dma_start(
                            y.ap()[t * P:(t + 1) * P,
                                   co:co + 2 * TBS].rearrange(
                                "o (a b) -> o a b", a=TB),
                            yt[:])

    # Ideal is 704 (512-256 for mm1, 512-64 for mm2); the tile scheduler
    # interleaves a few groups at accumulation boundaries, breaking ~60.
    n = _dedupe_ldweights(nc, mybir)
    assert n >= 600, f"expected >=600 deduped ldweights, got {n}"
    nc.compile()
    return nc


def _maybe_enable_trace():
    """Register the axon NTFF profile hook (test/dev only, KERNEL_TRACE=1)."""
    try:
        import sys
        import types

        try:
            from antenv.axon_hooks import get_axon_ntff_profile_hook  # noqa: F401
        except ImportError:
            store = {"h": None}
            mod = types.ModuleType("antenv.axon_hooks")
            mod.set_axon_ntff_profile_hook = lambda h: store.__setitem__("h", h)
            mod.get_axon_ntff_profile_hook = lambda: store["h"]
            sys.modules["antenv.axon_hooks"] = mod
        from antenv.axon_hooks import set_axon_ntff_profile_hook
        from trn_agent_boot.trn_boot import _ntff_profile_via_ctypes

        set_axon_ntff_profile_hook(
            _ntff_profile_via_ctypes("/opt/axon/libaxon_pjrt.so"))
        import concourse.bass_utils as bass_utils

        bass_utils.upload_artifacts = lambda tmpdir: tmpdir
        return True
    except Exception as e:  # pragma: no cover - trace is best-effort
        print(f"trace setup failed: {e}")
        return False


def kernel(x, U_data, U_scale, U_zp, S_data, S_scale, S_zp,
           Vh_data, Vh_scale, Vh_zp, bias):
    global last_run

    trace = bool(os.environ.get("KERNEL_TRACE"))
    if trace:
        trace = _maybe_enable_trace()

    from concourse.bass_utils import run_bass_kernel_spmd

    x = np.asarray(x, dtype=np.float32)
    s_v = float(np.asarray(Vh_scale).reshape(-1)[0])
    s_u = float(np.asarray(U_scale).reshape(-1)[0])
    s_s = float(np.asarray(S_scale).reshape(-1)[0])
    zp_v = float(np.asarray(Vh_zp).reshape(-1)[0])
    zp_u = float(np.asarray(U_zp).reshape(-1)[0])
    zp_s = float(np.asarray(S_zp).reshape(-1)[0])

    # Host folds: zero-point subtract (exact in bf16 for 8-bit ints) and all
    # multiplicative scales into the per-rank vector sc.
    vhT = np.asarray(Vh_data, dtype=np.float32).T - zp_v          # [in, r]
    vhr = np.ascontiguousarray(
        vhT.reshape(KO, P, 2, TBS).transpose(1, 2, 0, 3)).astype(_BF16)
    uT = np.asarray(U_data, dtype=np.float32).T - zp_u            # [r, out]
    ur = np.ascontiguousarray(
        uT.reshape(RO, P, OUT_F).transpose(1, 0, 2)).astype(_BF16)
    scv = ((np.asarray(S_data, dtype=np.float32) - zp_s)
           * (s_v * s_s * s_u)).reshape(RO, P).T                  # [P, RO]
    sc = np.ascontiguousarray(scv, dtype=np.float32)
    bias_np = np.asarray(bias, dtype=np.float32)

    nc = _nc_cache.get("nc")
    if nc is None:
        nc = _nc_cache["nc"] = _build_nc()

    x_bf = x.reshape(NTOK, IN_F).astype(_BF16)
    in_maps = []
    for c in range(N_CORES):
        xc = x_bf[c * TPC:(c + 1) * TPC]                          # [1024, 4096]
        # xr[p, ko, blk, t] = xc[blk*512 + t, ko*128 + p]
        xrc = np.ascontiguousarray(
            xc.reshape(TB, TBS, KO, P).transpose(3, 2, 0, 1))
        in_maps.append({
            "xr": xrc,
            "vhr": vhr,
            "ur": ur,
            "sc": sc,
        })

    kwargs = {}
    if trace:
        kwargs = dict(trace=True, tmpdir=os.environ.get("KERNEL_TRACE_DIR"))
        if os.environ.get("KERNEL_TRACE_ALL"):
            kwargs["trace_cores"] = list(range(N_CORES))
    res = run_bass_kernel_spmd(nc, in_maps, core_ids=list(range(N_CORES)),
                               **kwargs)
    last_run = res

    y = np.concatenate([res.results[c]["y"] for c in range(N_CORES)],
                       axis=0).astype(np.float32)
    y += bias_np[None, :]
    return y.reshape(B, S_LEN, OUT_F)
